# revision 21
# baseline (speedup 1.0000x reference)
"""GCN (4x GCNConv + global mean pool + MLP head) on 8 Trainium2 NeuronCores.

Key algebraic restructuring (vs. naive 4x gather-aggregate):
  - Layer 1 aggregation is linear in the static input:  agg0 = (A~ x) @ W1,
    so A~ x is precomputed on the host and layer 1 on device is one dense
    matmul + ReLU. No exchange, no gathers.
  - Layer 4 has no ReLU between aggregation and mean-pool, so the pool
    collapses through the aggregation: pooled[g] = sum_m C[m,g] * h4[m]
    with C[m,g] = sum_{e: src=m, graph(dst)=g} w_e + selfw_m [graph(m)=g],
    host-built. No exchange, no gathers, aggregation rides the pool matmul;
    cross-core reduction is the existing AllReduce.
  - Only layers 2 and 3 exchange features: h (fp8) is AllGathered per slab,
    edge rows are fetched with per-(window,subtile) indirect DMAs, and the
    aggregation runs as fp8 DoubleRow matmuls against host-built fp8
    selection/scale matrices S resident in SBUF (edge + self-loop diag).

Node sharding: 100k nodes -> 8 cores x 98 windows x 128 slots via
capacity-constrained bin packing (<=512 in-edges per window, so exactly
4 edge subtiles per window). All feature exchange in fp8 e4m3 with f32
accumulation; weights/activations in bf16.
"""

import heapq

import numpy as np
import ml_dtypes

# ---------------------------------------------------------------- constants
N_NODES = 100000
N_EDGES = 400000
N_GRAPHS = 64
DIMS = [(512, 512), (512, 256), (256, 128), (128, 64)]
N_CORES = 8
P = 128
W_WINDOWS = 98
SLOTS = W_WINDOWS * P            # 12544 per core
SLOTS_ALL = SLOTS * N_CORES      # 100352
BF16 = ml_dtypes.bfloat16
F8 = ml_dtypes.float8_e4m3fn
SUB = 4                          # edge subtiles per window
NSUB = W_WINDOWS * SUB           # 392 per core
N_SLAB = 4                       # AllGather slabs per exchanged layer


def _slabs(w_windows):
    base = w_windows // N_SLAB
    rem = w_windows % N_SLAB
    out = []
    w0 = 0
    for s in range(N_SLAB):
        nwin = base + (1 if s < rem else 0)
        if nwin > 0:
            out.append((w0, nwin))
        w0 += nwin
    return out


# ---------------------------------------------------------------- host prep
def _pack_nodes(cost, sub_real):
    nb = N_CORES * W_WINDOWS
    cap = sub_real * P
    order = np.argsort(-cost, kind="stable")
    bin_load = np.zeros(nb, dtype=np.int64)
    bin_cnt = np.zeros(nb, dtype=np.int64)
    node_bin = np.full(len(cost), -1, dtype=np.int64)
    heap = [(0, b) for b in range(nb)]
    heapq.heapify(heap)
    stash = []
    for n in order:
        c = cost[n]
        stash.clear()
        placed = False
        while heap:
            load, b = heapq.heappop(heap)
            if bin_load[b] + c <= cap and bin_cnt[b] < P:
                bin_load[b] += c
                bin_cnt[b] += 1
                node_bin[n] = b
                if bin_cnt[b] < P:
                    heapq.heappush(heap, (bin_load[b], b))
                placed = True
                break
            elif bin_cnt[b] < P:
                stash.append((load, b))
        for it in stash:
            heapq.heappush(heap, it)
        if not placed:
            return None, None
    return node_bin, bin_load


def _preprocess(x, edge_index, batch):
    src = np.asarray(edge_index[0], dtype=np.int64)
    dst = np.asarray(edge_index[1], dtype=np.int64)
    batch = np.asarray(batch, dtype=np.int64)
    n = x.shape[0]

    indeg = np.bincount(dst, minlength=n).astype(np.int64)
    deg = indeg.astype(np.float64) + 1.0
    dinv = 1.0 / np.sqrt(deg)
    enorm = (dinv[src] * dinv[dst]).astype(np.float32)
    selfw = (dinv * dinv).astype(np.float32)

    node_bin, bin_load = _pack_nodes(indeg, SUB)
    assert node_bin is not None, "window packing failed at cap 512"

    nb = N_CORES * W_WINDOWS
    order = np.argsort(-bin_load, kind="stable")
    bin_core = np.empty(nb, dtype=np.int64)
    bin_win = np.empty(nb, dtype=np.int64)
    for i, b in enumerate(order):
        rnd, k = divmod(i, N_CORES)
        c = k if rnd % 2 == 0 else N_CORES - 1 - k
        bin_core[b] = c
        bin_win[b] = rnd

    node_core = bin_core[node_bin]
    node_win = bin_win[node_bin]
    gkey = node_core * W_WINDOWS + node_win
    sort_idx = np.argsort(gkey, kind="stable")
    gsorted = gkey[sort_idx]
    grp_start = np.searchsorted(gsorted, np.arange(nb))
    slot_in_win = np.empty(n, dtype=np.int64)
    slot_in_win[sort_idx] = np.arange(n) - grp_start[gsorted]
    assert slot_in_win.max() < P
    node_slot = node_core * SLOTS + node_win * P + slot_in_win

    # agout global row (slab-major layout so AG slabs are contiguous)
    slabs = _slabs(W_WINDOWS)
    win_slab = np.zeros(W_WINDOWS, dtype=np.int64)
    win_off = np.zeros(W_WINDOWS, dtype=np.int64)
    slab_off = np.zeros(N_SLAB, dtype=np.int64)
    slab_rows = np.zeros(N_SLAB, dtype=np.int64)
    off = 0
    for s, (w0, nwin) in enumerate(slabs):
        win_slab[w0:w0 + nwin] = s
        win_off[w0:w0 + nwin] = np.arange(nwin)
        slab_off[s] = off
        slab_rows[s] = nwin * P
        off += N_CORES * nwin * P
    node_grow = (slab_off[win_slab[node_win]]
                 + node_core * slab_rows[win_slab[node_win]]
                 + win_off[node_win] * P + slot_in_win)

    # ---- edge slot layout: per (core, window) 4 subtiles of 128
    e_core = node_core[dst]
    e_win = node_win[dst]
    e_pdst = slot_in_win[dst]

    ekey = e_core * W_WINDOWS + e_win
    es = np.argsort(ekey, kind="stable")
    eks = ekey[es]
    egrp_start = np.searchsorted(eks, np.arange(nb))
    e_rank = np.empty(len(ekey), dtype=np.int64)
    e_rank[es] = np.arange(len(ekey)) - egrp_start[eks]
    assert e_rank.max() < SUB * P, "window overflow"

    e_j = e_rank // P
    e_p = e_rank % P

    gidx = np.zeros((N_CORES, P, NSUB), dtype=np.int32)
    gidx[e_core, e_p, e_win * SUB + e_j] = node_grow[src].astype(np.int32)

    # host-built fp8 S tiles: edge subtiles + self-loop diagonals
    S8 = np.zeros((N_CORES, P, NSUB, P), dtype=np.float32)
    S8[e_core, e_p, e_win * SUB + e_j, e_pdst] = enorm
    S8 = S8.astype(F8).reshape(N_CORES, P, NSUB * P)
    selfS8 = np.zeros((N_CORES, P, W_WINDOWS, P), dtype=np.float32)
    selfS8[node_core, slot_in_win, node_win, slot_in_win] = selfw
    selfS8 = selfS8.astype(F8).reshape(N_CORES, P, W_WINDOWS * P)

    # ---- layer-1 host aggregation: X~ = A~ x  (then device does X~ @ W1)
    xt = x.astype(np.float32)
    ax = selfw[:, None] * xt
    np.add.at(ax, dst, enorm[:, None] * xt[src])
    x0 = np.zeros((N_CORES, SLOTS, xt.shape[1]), dtype=BF16)
    x0.reshape(SLOTS_ALL, -1)[node_slot] = ax.astype(BF16)
    x0 = np.ascontiguousarray(x0.transpose(0, 2, 1))

    # ---- layer-4 aggregation folded into the pool matrix:
    #   C[m, g] = sum_{e: src=m, graph(dst)=g} w_e + selfw_m [graph(m)=g]
    Cm = np.zeros((n, N_GRAPHS), dtype=np.float32)
    np.add.at(Cm, (src, batch[dst]), enorm)
    Cm[np.arange(n), batch] += selfw
    poolC = np.zeros((N_CORES, P, W_WINDOWS * N_GRAPHS), dtype=BF16)
    pc = (node_win * N_GRAPHS)[:, None] + np.arange(N_GRAPHS)[None, :]
    poolC[node_core[:, None], slot_in_win[:, None], pc] = Cm.astype(BF16)

    cnts = np.bincount(batch, minlength=N_GRAPHS).astype(np.float32)
    inv_cnt = (1.0 / np.maximum(cnts, 1.0)).reshape(N_GRAPHS, 1)

    return dict(gidx=gidx, S8=S8, selfS8=selfS8, x0=x0,
                poolC=poolC, inv_cnt=inv_cnt)


# ---------------------------------------------------------------- device IR
def build_program(has_bias, n_cores=N_CORES, w_windows=W_WINDOWS,
                  dims=DIMS, n_graphs=N_GRAPHS):
    from contextlib import ExitStack

    import concourse.bass as bass
    import concourse.tile as tile
    from concourse import bacc, mybir
    from concourse.masks import make_identity

    dt = mybir.dt
    f32, bf16, i32, f8 = dt.float32, dt.bfloat16, dt.int32, dt.float8e4
    AF = mybir.ActivationFunctionType
    ALU = mybir.AluOpType
    PM = mybir.MatmulPerfMode
    W = w_windows
    slots = W * P
    slots_all = slots * n_cores
    G = n_graphs
    d_last = dims[-1][1]
    nlay = len(dims)
    rg = [list(range(n_cores))]
    XL = (1, 2)  # layers whose aggregation needs exchange + gathers
    assert not has_bias[3], "C-pool fold requires b4 == 0"

    nc = bacc.Bacc("TRN2", target_bir_lowering=False, debug=False,
                   num_devices=n_cores)

    # ---- I/O
    x0 = nc.dram_tensor("x0", [dims[0][0], slots], bf16, kind="ExternalInput")
    gidx_d = nc.dram_tensor("gidx", [P, NSUB], i32, kind="ExternalInput")
    S_d = nc.dram_tensor("S8", [P, NSUB * P], f8, kind="ExternalInput")
    selfS_d = nc.dram_tensor("selfS8", [P, W * P], f8, kind="ExternalInput")
    W_d = [nc.dram_tensor(f"W{i+1}", [di, do], bf16, kind="ExternalInput")
           for i, (di, do) in enumerate(dims)]
    B_d = [nc.dram_tensor(f"B{i+1}", [P, do], f32, kind="ExternalInput")
           if has_bias[i] else None for i, (_, do) in enumerate(dims)]
    poolC_d = nc.dram_tensor("poolC", [P, W * G], bf16, kind="ExternalInput")
    Wl1_d = nc.dram_tensor("Wl1", [d_last, 32], f32, kind="ExternalInput")
    bl1_d = nc.dram_tensor("bl1", [32, 1], f32, kind="ExternalInput")
    Wl_d = nc.dram_tensor("Wl", [32, 2], f32, kind="ExternalInput")
    bl_d = nc.dram_tensor("bl", [2, 1], f32, kind="ExternalInput")
    invc_d = nc.dram_tensor("invc", [G, 1], f32, kind="ExternalInput")
    out_head = nc.dram_tensor("out_head", [2, G], f32, kind="ExternalOutput")

    # ---- internal DRAM (exchange only for layers in XL)
    agin = {l: nc.dram_tensor(f"agin{l}", [slots, dims[l][1]], f8)
            for l in XL}
    agout = {l: nc.dram_tensor(f"agout{l}", [slots_all, dims[l][1]], f8,
                               addr_space="Shared") for l in XL}
    feat = [nc.dram_tensor(f"feat{l}", [slots, do], bf16)
            for l, (_, do) in enumerate(dims[:-1])]
    pool_in = nc.dram_tensor("pool_in", [G, d_last], f32)
    pool_out = nc.dram_tensor("pool_out", [G, d_last], f32,
                              addr_space="Shared")

    with tile.TileContext(nc) as tc, ExitStack() as ctx:
        const = ctx.enter_context(tc.tile_pool(name="const", bufs=1))
        xt_pool = ctx.enter_context(tc.tile_pool(name="xt", bufs=8))
        g_pool = ctx.enter_context(tc.tile_pool(name="g", bufs=24))
        sg_pool = ctx.enter_context(tc.tile_pool(name="sg", bufs=8))
        pp_pool = ctx.enter_context(tc.tile_pool(name="pp", bufs=4))
        h_pool = ctx.enter_context(tc.tile_pool(name="h", bufs=8))
        psum_m = ctx.enter_context(tc.tile_pool(name="pm", bufs=3, space="PSUM"))
        psum_a = ctx.enter_context(tc.tile_pool(name="pa", bufs=4, space="PSUM"))
        psum_s = ctx.enter_context(tc.tile_pool(name="ps", bufs=1, space="PSUM"))

        # resident constants
        gidx_sb = const.tile([P, NSUB], i32, name="gidx_sb")
        nc.sync.dma_start(gidx_sb[:], gidx_d.ap())
        S_sb = const.tile([P, NSUB, P], f8, name="S_sb")
        nc.sync.dma_start(S_sb[:], S_d.ap().rearrange("p (q m) -> p q m", m=P))
        selfS_sb = const.tile([P, W, P], f8, name="selfS_sb")
        nc.sync.dma_start(selfS_sb[:],
                          selfS_d.ap().rearrange("p (w m) -> p w m", m=P))

        W_sb = []
        for l, (di, do) in enumerate(dims):
            ks = di // P
            t = const.tile([P, ks, do], bf16, name=f"W{l}_sb")
            nc.sync.dma_start(t[:], W_d[l].ap().rearrange(
                "(kt p) do -> p kt do", p=P))
            W_sb.append(t)
        B_sb = []
        for l, (_, do) in enumerate(dims):
            if has_bias[l]:
                t = const.tile([P, do], f32, name=f"B{l}_sb")
                nc.sync.dma_start(t[:], B_d[l].ap())
                B_sb.append(t)
            else:
                B_sb.append(None)

        Wl1_sb = const.tile([d_last, 32], f32, name="Wl1_sb")
        nc.sync.dma_start(Wl1_sb[:], Wl1_d.ap())
        bl1_sb = const.tile([32, 1], f32, name="bl1_sb")
        nc.sync.dma_start(bl1_sb[:], bl1_d.ap())
        Wl_sb = const.tile([32, 2], f32, name="Wl_sb")
        nc.sync.dma_start(Wl_sb[:], Wl_d.ap())
        bl_sb = const.tile([2, 1], f32, name="bl_sb")
        nc.sync.dma_start(bl_sb[:], bl_d.ap())
        invc_sb = const.tile([G, 1], f32, name="invc_sb")
        nc.sync.dma_start(invc_sb[:], invc_d.ap())
        iden = const.tile([G, G], f32, name="iden")
        make_identity(nc, iden[:])

        x0_v = x0.ap().rearrange("(kt p) s -> p kt s", p=P)
        slabs = _slabs(W)
        pool_ps = psum_s.tile([G, d_last], f32, name="pool_ps",
                              tag="ps_small")

        # ---- M: compute h_l[w] = feat[l-1][w] @ W_l
        # l == 0: X~ @ W1, straight to feat[0] (aggregation precomputed)
        # l in XL: cast to fp8, store to agin[l] for the exchange
        # l == 3: matmul C-pool immediately (aggregation folded into pool)
        def emit_m_window(l, w):
            di, do = dims[l]
            ks = di // P
            xt = xt_pool.tile([P, ks, P], bf16, tag="xt")
            if l == 0:
                nc.sync.dma_start(xt[:], x0_v[:, :, w * P:(w + 1) * P])
            else:
                nc.scalar.dma_start_transpose(
                    xt[:], feat[l - 1].ap()[w * P:(w + 1) * P, :])
            ps = psum_m.tile([P, do], f32, tag="pm")
            for kt in range(ks):
                nc.tensor.matmul(ps[:], lhsT=xt[:, kt, :],
                                 rhs=W_sb[l][:, kt, :],
                                 start=(kt == 0), stop=(kt == ks - 1))
            if l == 0:
                if has_bias[0]:
                    nc.vector.tensor_tensor(out=ps[:], in0=ps[:],
                                            in1=B_sb[0][:], op=ALU.add)
                ft = h_pool.tile([P, do], bf16, tag="ft0")
                nc.scalar.activation(ft[:], ps[:], AF.Relu)
                nc.scalar.dma_start(feat[0].ap()[w * P:(w + 1) * P, :], ft[:])
            elif l in XL:
                hm = h_pool.tile([P, do], f8, tag="hm")
                nc.vector.tensor_copy(hm[:], ps[:])
                nc.scalar.dma_start(agin[l].ap()[w * P:(w + 1) * P, :], hm[:])
            else:  # l == 3: h4 window -> C-pool matmul (requires b4 == 0)
                h4 = h_pool.tile([P, do], bf16, tag="h4")
                nc.vector.tensor_copy(h4[:], ps[:])
                pt = pp_pool.tile([P, G], bf16, tag="pp")
                nc.scalar.dma_start(
                    pt[:], poolC_d.ap()[:, w * G:(w + 1) * G])
                nc.tensor.matmul(pool_ps[:], lhsT=pt[:], rhs=h4[:],
                                 start=(w == 0), stop=(w == W - 1))

        def emit_ag_slab(l, w0s, nwin, goff):
            rows = nwin * P
            nc.gpsimd.collective_compute(
                "AllGather", mybir.AluOpType.bypass, replica_groups=rg,
                ins=[agin[l].ap()[w0s * P:w0s * P + rows, :]],
                outs=[agout[l].ap()[goff:goff + n_cores * rows, :]])

        goffs = []
        goff = 0
        for (w0s, nwin) in slabs:
            goffs.append(goff)
            goff += n_cores * nwin * P
        slab_last = {w0s + nwin - 1: (si, w0s, nwin)
                     for si, (w0s, nwin) in enumerate(slabs)}

        def emit_m_and_ag(l1, wm):
            emit_m_window(l1, wm)
            if l1 in XL and wm in slab_last:
                si, w0s, nwin = slab_last[wm]
                emit_ag_slab(l1, w0s, nwin, goffs[si])

        # ---- A: aggregate layer l in XL (gather + fp8 DoubleRow matmuls)
        gpend = {}

        def emit_gathers(l, w):
            do = dims[l][1]
            g = g_pool.tile([P, SUB, do], f8, tag="g")
            for j in range(SUB):
                nc.gpsimd.indirect_dma_start(
                    out=g[:, j, :], out_offset=None, in_=agout[l].ap(),
                    in_offset=bass.IndirectOffsetOnAxis(
                        ap=gidx_sb[:, w * SUB + j:w * SUB + j + 1],
                        axis=0))
            gs = sg_pool.tile([P, do], f8, tag="gs")
            nc.sync.dma_start(gs[:], agin[l].ap()[w * P:(w + 1) * P, :])
            gpend[(l, w)] = (g, gs)

        def emit_a_window(l, w):
            do = dims[l][1]
            g, gs = gpend.pop((l, w))
            ps = psum_a.tile([P, do], f32, tag="pa")
            for k in range(2):
                q = w * SUB + 2 * k
                nc.tensor.matmul(ps[:], lhsT=S_sb[:, q:q + 2, :],
                                 rhs=g[:, 2 * k:2 * k + 2, :],
                                 start=(k == 0), stop=False,
                                 perf_mode=PM.DoubleRow)
            nc.tensor.matmul(ps[:], lhsT=selfS_sb[:, w, :], rhs=gs[:],
                             start=False, stop=True)
            if has_bias[l]:
                nc.vector.tensor_tensor(out=ps[:], in0=ps[:],
                                        in1=B_sb[l][:], op=ALU.add)
            ft = h_pool.tile([P, do], bf16, tag=f"ft{do}")
            nc.scalar.activation(ft[:], ps[:], AF.Relu)
            nc.scalar.dma_start(feat[l].ap()[w * P:(w + 1) * P, :], ft[:])

        # ---------------- schedule
        # L0 (dense) with M(1) + AG(1) trailing right behind it
        LAG = 4
        PF = 5  # gather prefetch depth (windows)
        for w in range(W):
            emit_m_window(0, w)
            if w >= LAG:
                emit_m_and_ag(1, w - LAG)
        for wm in range(W - LAG, W):
            emit_m_and_ag(1, wm)
        # A(1) with M(2)/AG(2) trailing; A(2) with M(3)+pool trailing
        for l in XL:
            for k in range(PF):
                emit_gathers(l, k)
            for w in range(W):
                emit_a_window(l, w)
                if w + PF < W:
                    emit_gathers(l, w + PF)
                if w >= LAG:
                    emit_m_and_ag(l + 1, w - LAG)
            for wm in range(W - LAG, W):
                emit_m_and_ag(l + 1, wm)

        # ---------------- mean pool tail + head
        pool_sb = const.tile([G, d_last], f32, name="pool_sb")
        nc.vector.tensor_copy(pool_sb[:], pool_ps[:])
        nc.sync.dma_start(pool_in.ap(), pool_sb[:])
        nc.gpsimd.collective_compute(
            "AllReduce", mybir.AluOpType.add, replica_groups=rg,
            ins=[pool_in.ap()], outs=[pool_out.ap()])
        psum_sb = const.tile([G, d_last], f32, name="psum_sb")
        nc.sync.dma_start(psum_sb[:], pool_out.ap())
        pooled = const.tile([G, d_last], f32, name="pooled")
        nc.vector.tensor_scalar_mul(pooled[:], psum_sb[:], invc_sb[:, :1])

        pt_ps = psum_s.tile([d_last, G], f32, name="pt_ps", tag="ps_small")
        nc.tensor.transpose(pt_ps[:], pooled[:], iden[:])
        pt = const.tile([d_last, G], f32, name="pt")
        nc.vector.tensor_copy(pt[:], pt_ps[:])
        ps1 = psum_s.tile([32, G], f32, name="ps1", tag="ps_small")
        nc.tensor.matmul(ps1[:], lhsT=Wl1_sb[:], rhs=pt[:])
        h1 = const.tile([32, G], f32, name="h1")
        nc.scalar.activation(h1[:], ps1[:], AF.Relu, bias=bl1_sb[:, :1])
        ps2 = psum_s.tile([2, G], f32, name="ps2", tag="ps_small")
        nc.tensor.matmul(ps2[:], lhsT=Wl_sb[:], rhs=h1[:])
        oh = const.tile([2, G], f32, name="oh")
        nc.vector.tensor_scalar_add(oh[:], ps2[:], bl_sb[:, :1])
        nc.sync.dma_start(out_head.ap(), oh[:])

    nc.compile()
    return nc


# ---------------------------------------------------------------- entry
_CACHE = {}


def _make_in_maps(prep, inp):
    Ws = [np.asarray(inp[f"W{i+1}"]) for i in range(4)]
    bs = [np.asarray(inp[f"b{i+1}"]) for i in range(4)]
    has_bias = tuple(bool(np.any(b != 0)) for b in bs)
    in_maps = []
    for c in range(N_CORES):
        m = dict(
            x0=prep["x0"][c],
            gidx=prep["gidx"][c], S8=prep["S8"][c], selfS8=prep["selfS8"][c],
            poolC=prep["poolC"][c], invc=prep["inv_cnt"],
            Wl1=np.asarray(inp["Wl1"], np.float32),
            bl1=np.asarray(inp["bl1"], np.float32).reshape(-1, 1),
            Wl=np.asarray(inp["Wl"], np.float32),
            bl=np.asarray(inp["bl"], np.float32).reshape(-1, 1),
        )
        for i, (wm, bv) in enumerate(zip(Ws, bs)):
            m[f"W{i+1}"] = wm.astype(BF16)
            if has_bias[i]:
                m[f"B{i+1}"] = np.broadcast_to(
                    np.asarray(bv, np.float32), (P, len(bv))).copy()
        in_maps.append(m)
    return in_maps


def kernel(x, edge_index, batch, W1, b1, W2, b2, W3, b3, W4, b4,
           Wl1, bl1, Wl, bl):
    from concourse import bass_utils

    x = np.asarray(x)
    prep = _preprocess(x, np.asarray(edge_index), np.asarray(batch))
    bs = [np.asarray(b) for b in (b1, b2, b3, b4)]
    has_bias = tuple(bool(np.any(b != 0)) for b in bs)

    if has_bias not in _CACHE:
        _CACHE[has_bias] = build_program(has_bias)
    nc = _CACHE[has_bias]

    inp = dict(W1=W1, b1=b1, W2=W2, b2=b2, W3=W3, b3=b3, W4=W4, b4=b4,
               Wl1=Wl1, bl1=bl1, Wl=Wl, bl=bl)
    in_maps = _make_in_maps(prep, inp)
    res = bass_utils.run_bass_kernel_spmd(
        nc, in_maps, core_ids=list(range(N_CORES)))
    out = res.results[0]["out_head"]
    return np.ascontiguousarray(out.T.astype(np.float32))


# revision 22
# speedup vs baseline: 1.1894x; 1.1894x over previous
"""GCN (4x GCNConv + global mean pool + MLP head) on 8 Trainium2 NeuronCores.

Key algebraic restructuring (vs. naive 4x gather-aggregate):
  - Layer 1 aggregation is linear in the static input:  agg0 = (A~ x) @ W1,
    so A~ x is precomputed on the host and layer 1 on device is one dense
    matmul + ReLU. No exchange, no gathers.
  - Layer 4 has no ReLU between aggregation and mean-pool, so the pool
    collapses through the aggregation: pooled[g] = sum_m C[m,g] * h4[m]
    with C[m,g] = sum_{e: src=m, graph(dst)=g} w_e + selfw_m [graph(m)=g],
    host-built. No exchange, no gathers, aggregation rides the pool matmul;
    cross-core reduction is the existing AllReduce.
  - Only layers 2 and 3 exchange features: h (fp8) is AllGathered per slab,
    edge rows are fetched with per-(window,subtile) indirect DMAs, and the
    aggregation runs as fp8 DoubleRow matmuls against host-built fp8
    selection/scale matrices S resident in SBUF (edge + self-loop diag).

Node sharding: 100k nodes -> 8 cores x 98 windows x 128 slots via
capacity-constrained bin packing (<=512 in-edges per window, so exactly
4 edge subtiles per window). All feature exchange in fp8 e4m3 with f32
accumulation; weights/activations in bf16.
"""

import heapq

import numpy as np
import ml_dtypes

# ---------------------------------------------------------------- constants
N_NODES = 100000
N_EDGES = 400000
N_GRAPHS = 64
DIMS = [(512, 512), (512, 256), (256, 128), (128, 64)]
N_CORES = 8
P = 128
W_WINDOWS = 98
SLOTS = W_WINDOWS * P            # 12544 per core
SLOTS_ALL = SLOTS * N_CORES      # 100352
BF16 = ml_dtypes.bfloat16
F8 = ml_dtypes.float8_e4m3fn
SUB = 4                          # edge subtiles per window
NSUB = W_WINDOWS * SUB           # 392 per core
N_SLAB = 4                       # AllGather slabs per exchanged layer


def _slabs(w_windows):
    base = w_windows // N_SLAB
    rem = w_windows % N_SLAB
    out = []
    w0 = 0
    for s in range(N_SLAB):
        nwin = base + (1 if s < rem else 0)
        if nwin > 0:
            out.append((w0, nwin))
        w0 += nwin
    return out


# ---------------------------------------------------------------- host prep
def _pack_nodes(cost, sub_real):
    nb = N_CORES * W_WINDOWS
    cap = sub_real * P
    order = np.argsort(-cost, kind="stable")
    bin_load = np.zeros(nb, dtype=np.int64)
    bin_cnt = np.zeros(nb, dtype=np.int64)
    node_bin = np.full(len(cost), -1, dtype=np.int64)
    heap = [(0, b) for b in range(nb)]
    heapq.heapify(heap)
    stash = []
    for n in order:
        c = cost[n]
        stash.clear()
        placed = False
        while heap:
            load, b = heapq.heappop(heap)
            if bin_load[b] + c <= cap and bin_cnt[b] < P:
                bin_load[b] += c
                bin_cnt[b] += 1
                node_bin[n] = b
                if bin_cnt[b] < P:
                    heapq.heappush(heap, (bin_load[b], b))
                placed = True
                break
            elif bin_cnt[b] < P:
                stash.append((load, b))
        for it in stash:
            heapq.heappush(heap, it)
        if not placed:
            return None, None
    return node_bin, bin_load


def _preprocess(x, edge_index, batch):
    src = np.asarray(edge_index[0], dtype=np.int64)
    dst = np.asarray(edge_index[1], dtype=np.int64)
    batch = np.asarray(batch, dtype=np.int64)
    n = x.shape[0]

    indeg = np.bincount(dst, minlength=n).astype(np.int64)
    deg = indeg.astype(np.float64) + 1.0
    dinv = 1.0 / np.sqrt(deg)
    enorm = (dinv[src] * dinv[dst]).astype(np.float32)
    selfw = (dinv * dinv).astype(np.float32)

    node_bin, bin_load = _pack_nodes(indeg, SUB)
    assert node_bin is not None, "window packing failed at cap 512"

    nb = N_CORES * W_WINDOWS
    order = np.argsort(-bin_load, kind="stable")
    bin_core = np.empty(nb, dtype=np.int64)
    bin_win = np.empty(nb, dtype=np.int64)
    for i, b in enumerate(order):
        rnd, k = divmod(i, N_CORES)
        c = k if rnd % 2 == 0 else N_CORES - 1 - k
        bin_core[b] = c
        bin_win[b] = rnd

    node_core = bin_core[node_bin]
    node_win = bin_win[node_bin]
    gkey = node_core * W_WINDOWS + node_win
    sort_idx = np.argsort(gkey, kind="stable")
    gsorted = gkey[sort_idx]
    grp_start = np.searchsorted(gsorted, np.arange(nb))
    slot_in_win = np.empty(n, dtype=np.int64)
    slot_in_win[sort_idx] = np.arange(n) - grp_start[gsorted]
    assert slot_in_win.max() < P
    node_slot = node_core * SLOTS + node_win * P + slot_in_win

    # agout global row (slab-major layout so AG slabs are contiguous)
    slabs = _slabs(W_WINDOWS)
    win_slab = np.zeros(W_WINDOWS, dtype=np.int64)
    win_off = np.zeros(W_WINDOWS, dtype=np.int64)
    slab_off = np.zeros(N_SLAB, dtype=np.int64)
    slab_rows = np.zeros(N_SLAB, dtype=np.int64)
    off = 0
    for s, (w0, nwin) in enumerate(slabs):
        win_slab[w0:w0 + nwin] = s
        win_off[w0:w0 + nwin] = np.arange(nwin)
        slab_off[s] = off
        slab_rows[s] = nwin * P
        off += N_CORES * nwin * P
    node_grow = (slab_off[win_slab[node_win]]
                 + node_core * slab_rows[win_slab[node_win]]
                 + win_off[node_win] * P + slot_in_win)

    # ---- edge slot layout: per (core, window) 4 subtiles of 128
    e_core = node_core[dst]
    e_win = node_win[dst]
    e_pdst = slot_in_win[dst]

    ekey = e_core * W_WINDOWS + e_win
    es = np.argsort(ekey, kind="stable")
    eks = ekey[es]
    egrp_start = np.searchsorted(eks, np.arange(nb))
    e_rank = np.empty(len(ekey), dtype=np.int64)
    e_rank[es] = np.arange(len(ekey)) - egrp_start[eks]
    assert e_rank.max() < SUB * P, "window overflow"

    e_j = e_rank // P
    e_p = e_rank % P

    gidx = np.zeros((N_CORES, P, NSUB), dtype=np.int32)
    gidx[e_core, e_p, e_win * SUB + e_j] = node_grow[src].astype(np.int32)

    # host-built fp8 S tiles: edge subtiles + self-loop diagonals
    S8 = np.zeros((N_CORES, P, NSUB, P), dtype=np.float32)
    S8[e_core, e_p, e_win * SUB + e_j, e_pdst] = enorm
    S8 = S8.astype(F8).reshape(N_CORES, P, NSUB * P)
    selfS8 = np.zeros((N_CORES, P, W_WINDOWS, P), dtype=np.float32)
    selfS8[node_core, slot_in_win, node_win, slot_in_win] = selfw
    selfS8 = selfS8.astype(F8).reshape(N_CORES, P, W_WINDOWS * P)

    # ---- layer-1 host aggregation: X~ = A~ x  (then device does X~ @ W1)
    xt = x.astype(np.float32)
    ax = selfw[:, None] * xt
    np.add.at(ax, dst, enorm[:, None] * xt[src])
    x0 = np.zeros((N_CORES, SLOTS, xt.shape[1]), dtype=BF16)
    x0.reshape(SLOTS_ALL, -1)[node_slot] = ax.astype(BF16)
    x0 = np.ascontiguousarray(x0.transpose(0, 2, 1))

    # ---- layer-4 aggregation folded into the pool matrix:
    #   C[m, g] = sum_{e: src=m, graph(dst)=g} w_e + selfw_m [graph(m)=g]
    Cm = np.zeros((n, N_GRAPHS), dtype=np.float32)
    np.add.at(Cm, (src, batch[dst]), enorm)
    Cm[np.arange(n), batch] += selfw
    poolC = np.zeros((N_CORES, P, W_WINDOWS * N_GRAPHS), dtype=BF16)
    pc = (node_win * N_GRAPHS)[:, None] + np.arange(N_GRAPHS)[None, :]
    poolC[node_core[:, None], slot_in_win[:, None], pc] = Cm.astype(BF16)

    cnts = np.bincount(batch, minlength=N_GRAPHS).astype(np.float32)
    inv_cnt = (1.0 / np.maximum(cnts, 1.0)).reshape(N_GRAPHS, 1)

    return dict(gidx=gidx, S8=S8, selfS8=selfS8, x0=x0,
                poolC=poolC, inv_cnt=inv_cnt)


# ---------------------------------------------------------------- device IR
def build_program(has_bias, n_cores=N_CORES, w_windows=W_WINDOWS,
                  dims=DIMS, n_graphs=N_GRAPHS):
    from contextlib import ExitStack

    import concourse.bass as bass
    import concourse.tile as tile
    from concourse import bacc, mybir
    from concourse.masks import make_identity

    dt = mybir.dt
    f32, bf16, i32, f8 = dt.float32, dt.bfloat16, dt.int32, dt.float8e4
    AF = mybir.ActivationFunctionType
    ALU = mybir.AluOpType
    PM = mybir.MatmulPerfMode
    W = w_windows
    slots = W * P
    slots_all = slots * n_cores
    G = n_graphs
    d_last = dims[-1][1]
    nlay = len(dims)
    rg = [list(range(n_cores))]
    XL = (1, 2)  # layers whose aggregation needs exchange + gathers
    assert not has_bias[3], "C-pool fold requires b4 == 0"

    nc = bacc.Bacc("TRN2", target_bir_lowering=False, debug=False,
                   num_devices=n_cores)

    # ---- I/O
    x0 = nc.dram_tensor("x0", [dims[0][0], slots], bf16, kind="ExternalInput")
    gidx_d = nc.dram_tensor("gidx", [P, NSUB], i32, kind="ExternalInput")
    S_d = nc.dram_tensor("S8", [P, NSUB * P], f8, kind="ExternalInput")
    selfS_d = nc.dram_tensor("selfS8", [P, W * P], f8, kind="ExternalInput")
    W_d = [nc.dram_tensor(f"W{i+1}", [di, do], bf16, kind="ExternalInput")
           for i, (di, do) in enumerate(dims)]
    B_d = [nc.dram_tensor(f"B{i+1}", [P, do], f32, kind="ExternalInput")
           if has_bias[i] else None for i, (_, do) in enumerate(dims)]
    poolC_d = nc.dram_tensor("poolC", [P, W * G], bf16, kind="ExternalInput")
    Wl1_d = nc.dram_tensor("Wl1", [d_last, 32], f32, kind="ExternalInput")
    bl1_d = nc.dram_tensor("bl1", [32, 1], f32, kind="ExternalInput")
    Wl_d = nc.dram_tensor("Wl", [32, 2], f32, kind="ExternalInput")
    bl_d = nc.dram_tensor("bl", [2, 1], f32, kind="ExternalInput")
    invc_d = nc.dram_tensor("invc", [G, 1], f32, kind="ExternalInput")
    out_head = nc.dram_tensor("out_head", [2, G], f32, kind="ExternalOutput")

    # ---- internal DRAM (exchange only for layers in XL)
    agin = {l: nc.dram_tensor(f"agin{l}", [slots, dims[l][1]], f8)
            for l in XL}
    agout = {l: nc.dram_tensor(f"agout{l}", [slots_all, dims[l][1]], f8,
                               addr_space="Shared") for l in XL}
    feat = [nc.dram_tensor(f"feat{l}", [slots, do], bf16)
            for l, (_, do) in enumerate(dims[:-1])]
    pool_in = nc.dram_tensor("pool_in", [G, d_last], f32)
    pool_out = nc.dram_tensor("pool_out", [G, d_last], f32,
                              addr_space="Shared")

    with tile.TileContext(nc) as tc, ExitStack() as ctx:
        const = ctx.enter_context(tc.tile_pool(name="const", bufs=1))
        xt_pool = ctx.enter_context(tc.tile_pool(name="xt", bufs=8))
        g_pool = ctx.enter_context(tc.tile_pool(name="g", bufs=24))
        sg_pool = ctx.enter_context(tc.tile_pool(name="sg", bufs=6))
        pp_pool = ctx.enter_context(tc.tile_pool(name="pp", bufs=4))
        h_pool = ctx.enter_context(tc.tile_pool(name="h", bufs=8))
        psum_m = ctx.enter_context(tc.tile_pool(name="pm", bufs=2, space="PSUM"))
        psum_a = ctx.enter_context(tc.tile_pool(name="pa", bufs=4, space="PSUM"))
        psum_s = ctx.enter_context(tc.tile_pool(name="ps", bufs=2, space="PSUM"))

        # resident constants
        gidx_sb = const.tile([P, NSUB], i32, name="gidx_sb")
        nc.sync.dma_start(gidx_sb[:], gidx_d.ap())
        S_sb = const.tile([P, NSUB, P], f8, name="S_sb")
        nc.sync.dma_start(S_sb[:], S_d.ap().rearrange("p (q m) -> p q m", m=P))
        selfS_sb = const.tile([P, W, P], f8, name="selfS_sb")
        nc.sync.dma_start(selfS_sb[:],
                          selfS_d.ap().rearrange("p (w m) -> p w m", m=P))

        W_sb = []
        for l, (di, do) in enumerate(dims):
            ks = di // P
            t = const.tile([P, ks, do], bf16, name=f"W{l}_sb")
            nc.sync.dma_start(t[:], W_d[l].ap().rearrange(
                "(kt p) do -> p kt do", p=P))
            W_sb.append(t)
        B_sb = []
        for l, (_, do) in enumerate(dims):
            if has_bias[l]:
                t = const.tile([P, do], f32, name=f"B{l}_sb")
                nc.sync.dma_start(t[:], B_d[l].ap())
                B_sb.append(t)
            else:
                B_sb.append(None)

        Wl1_sb = const.tile([d_last, 32], f32, name="Wl1_sb")
        nc.sync.dma_start(Wl1_sb[:], Wl1_d.ap())
        bl1_sb = const.tile([32, 1], f32, name="bl1_sb")
        nc.sync.dma_start(bl1_sb[:], bl1_d.ap())
        Wl_sb = const.tile([32, 2], f32, name="Wl_sb")
        nc.sync.dma_start(Wl_sb[:], Wl_d.ap())
        bl_sb = const.tile([2, 1], f32, name="bl_sb")
        nc.sync.dma_start(bl_sb[:], bl_d.ap())
        invc_sb = const.tile([G, 1], f32, name="invc_sb")
        nc.sync.dma_start(invc_sb[:], invc_d.ap())
        iden = const.tile([G, G], f32, name="iden")
        make_identity(nc, iden[:])

        x0_v = x0.ap().rearrange("(kt p) s -> p kt s", p=P)
        slabs = _slabs(W)
        pool_ps = psum_s.tile([G, d_last], f32, name="pool_ps",
                              tag="ps_small")

        # ---- M: compute h_l[w] = feat[l-1][w] @ W_l
        # l == 0: X~ @ W1, straight to feat[0] (aggregation precomputed)
        # l in XL: cast to fp8, store to agin[l] for the exchange
        # l == 3: matmul C-pool immediately (aggregation folded into pool)
        def emit_m_window(l, w):
            di, do = dims[l]
            ks = di // P
            xt = xt_pool.tile([P, ks, P], bf16, tag="xt")
            if l == 0:
                nc.sync.dma_start(xt[:], x0_v[:, :, w * P:(w + 1) * P])
            else:
                nc.sync.dma_start_transpose(
                    xt[:], feat[l - 1].ap()[w * P:(w + 1) * P, :])
            ps = psum_m.tile([P, do], f32, tag="pm")
            for kt in range(ks):
                nc.tensor.matmul(ps[:], lhsT=xt[:, kt, :],
                                 rhs=W_sb[l][:, kt, :],
                                 start=(kt == 0), stop=(kt == ks - 1))
            if l == 0:
                if has_bias[0]:
                    nc.vector.tensor_tensor(out=ps[:], in0=ps[:],
                                            in1=B_sb[0][:], op=ALU.add)
                ft = h_pool.tile([P, do], bf16, tag="ft0")
                nc.scalar.activation(ft[:], ps[:], AF.Relu)
                nc.scalar.dma_start(feat[0].ap()[w * P:(w + 1) * P, :], ft[:])
            elif l in XL:
                hm = h_pool.tile([P, do], f8, tag="hm")
                nc.vector.tensor_copy(hm[:], ps[:])
                nc.scalar.dma_start(agin[l].ap()[w * P:(w + 1) * P, :], hm[:])
            else:  # l == 3: h4 window -> C-pool matmul (requires b4 == 0)
                h4 = h_pool.tile([P, do], bf16, tag="h4")
                nc.vector.tensor_copy(h4[:], ps[:])
                pt = pp_pool.tile([P, G], bf16, tag="pp")
                nc.scalar.dma_start(
                    pt[:], poolC_d.ap()[:, w * G:(w + 1) * G])
                nc.tensor.matmul(pool_ps[:], lhsT=pt[:], rhs=h4[:],
                                 start=(w == 0), stop=(w == W - 1))

        def emit_ag_slab(l, w0s, nwin, goff):
            rows = nwin * P
            nc.gpsimd.collective_compute(
                "AllGather", mybir.AluOpType.bypass, replica_groups=rg,
                ins=[agin[l].ap()[w0s * P:w0s * P + rows, :]],
                outs=[agout[l].ap()[goff:goff + n_cores * rows, :]])

        goffs = []
        goff = 0
        for (w0s, nwin) in slabs:
            goffs.append(goff)
            goff += n_cores * nwin * P
        slab_last = {w0s + nwin - 1: (si, w0s, nwin)
                     for si, (w0s, nwin) in enumerate(slabs)}

        def emit_m_and_ag(l1, wm):
            emit_m_window(l1, wm)
            if l1 in XL and wm in slab_last:
                si, w0s, nwin = slab_last[wm]
                emit_ag_slab(l1, w0s, nwin, goffs[si])

        # ---- A: aggregate layer l in XL (gather + fp8 DoubleRow matmuls)
        def emit_a_window(l, w):
            do = dims[l][1]
            g = g_pool.tile([P, SUB, do], f8, tag="g")
            for j in range(SUB):
                nc.gpsimd.indirect_dma_start(
                    out=g[:, j, :], out_offset=None, in_=agout[l].ap(),
                    in_offset=bass.IndirectOffsetOnAxis(
                        ap=gidx_sb[:, w * SUB + j:w * SUB + j + 1],
                        axis=0))
            gs = sg_pool.tile([P, do], f8, tag="gs")
            nc.sync.dma_start(gs[:], agin[l].ap()[w * P:(w + 1) * P, :])
            ps = psum_a.tile([P, do], f32, tag="pa")
            for k in range(2):
                q = w * SUB + 2 * k
                nc.tensor.matmul(ps[:], lhsT=S_sb[:, q:q + 2, :],
                                 rhs=g[:, 2 * k:2 * k + 2, :],
                                 start=(k == 0), stop=False,
                                 perf_mode=PM.DoubleRow)
            nc.tensor.matmul(ps[:], lhsT=selfS_sb[:, w, :], rhs=gs[:],
                             start=False, stop=True)
            if has_bias[l]:
                nc.vector.tensor_tensor(out=ps[:], in0=ps[:],
                                        in1=B_sb[l][:], op=ALU.add)
            ft = h_pool.tile([P, do], bf16, tag=f"ft{do}")
            nc.scalar.activation(ft[:], ps[:], AF.Relu)
            nc.scalar.dma_start(feat[l].ap()[w * P:(w + 1) * P, :], ft[:])

        # ---------------- schedule
        # L0 (dense) with M(1) + AG(1) trailing right behind it
        LAG = 2
        for w in range(W):
            emit_m_window(0, w)
            if w >= LAG:
                emit_m_and_ag(1, w - LAG)
        for wm in range(W - LAG, W):
            emit_m_and_ag(1, wm)
        # A(1) with M(2)/AG(2) trailing; A(2) with M(3)+pool trailing
        for l in XL:
            for w in range(W):
                emit_a_window(l, w)
                if w >= LAG:
                    emit_m_and_ag(l + 1, w - LAG)
            for wm in range(W - LAG, W):
                emit_m_and_ag(l + 1, wm)

        # ---------------- mean pool tail + head
        pool_sb = const.tile([G, d_last], f32, name="pool_sb")
        nc.vector.tensor_copy(pool_sb[:], pool_ps[:])
        nc.sync.dma_start(pool_in.ap(), pool_sb[:])
        nc.gpsimd.collective_compute(
            "AllReduce", mybir.AluOpType.add, replica_groups=rg,
            ins=[pool_in.ap()], outs=[pool_out.ap()])
        psum_sb = const.tile([G, d_last], f32, name="psum_sb")
        nc.sync.dma_start(psum_sb[:], pool_out.ap())
        pooled = const.tile([G, d_last], f32, name="pooled")
        nc.vector.tensor_scalar_mul(pooled[:], psum_sb[:], invc_sb[:, :1])

        pt_ps = psum_s.tile([d_last, G], f32, name="pt_ps", tag="ps_small")
        nc.tensor.transpose(pt_ps[:], pooled[:], iden[:])
        pt = const.tile([d_last, G], f32, name="pt")
        nc.vector.tensor_copy(pt[:], pt_ps[:])
        ps1 = psum_s.tile([32, G], f32, name="ps1", tag="ps_small")
        nc.tensor.matmul(ps1[:], lhsT=Wl1_sb[:], rhs=pt[:])
        h1 = const.tile([32, G], f32, name="h1")
        nc.scalar.activation(h1[:], ps1[:], AF.Relu, bias=bl1_sb[:, :1])
        ps2 = psum_s.tile([2, G], f32, name="ps2", tag="ps_small")
        nc.tensor.matmul(ps2[:], lhsT=Wl_sb[:], rhs=h1[:])
        oh = const.tile([2, G], f32, name="oh")
        nc.vector.tensor_scalar_add(oh[:], ps2[:], bl_sb[:, :1])
        nc.sync.dma_start(out_head.ap(), oh[:])

    nc.compile()
    return nc


# ---------------------------------------------------------------- entry
_CACHE = {}


def _make_in_maps(prep, inp):
    Ws = [np.asarray(inp[f"W{i+1}"]) for i in range(4)]
    bs = [np.asarray(inp[f"b{i+1}"]) for i in range(4)]
    has_bias = tuple(bool(np.any(b != 0)) for b in bs)
    in_maps = []
    for c in range(N_CORES):
        m = dict(
            x0=prep["x0"][c],
            gidx=prep["gidx"][c], S8=prep["S8"][c], selfS8=prep["selfS8"][c],
            poolC=prep["poolC"][c], invc=prep["inv_cnt"],
            Wl1=np.asarray(inp["Wl1"], np.float32),
            bl1=np.asarray(inp["bl1"], np.float32).reshape(-1, 1),
            Wl=np.asarray(inp["Wl"], np.float32),
            bl=np.asarray(inp["bl"], np.float32).reshape(-1, 1),
        )
        for i, (wm, bv) in enumerate(zip(Ws, bs)):
            m[f"W{i+1}"] = wm.astype(BF16)
            if has_bias[i]:
                m[f"B{i+1}"] = np.broadcast_to(
                    np.asarray(bv, np.float32), (P, len(bv))).copy()
        in_maps.append(m)
    return in_maps


def kernel(x, edge_index, batch, W1, b1, W2, b2, W3, b3, W4, b4,
           Wl1, bl1, Wl, bl):
    from concourse import bass_utils

    x = np.asarray(x)
    prep = _preprocess(x, np.asarray(edge_index), np.asarray(batch))
    bs = [np.asarray(b) for b in (b1, b2, b3, b4)]
    has_bias = tuple(bool(np.any(b != 0)) for b in bs)

    if has_bias not in _CACHE:
        _CACHE[has_bias] = build_program(has_bias)
    nc = _CACHE[has_bias]

    inp = dict(W1=W1, b1=b1, W2=W2, b2=b2, W3=W3, b3=b3, W4=W4, b4=b4,
               Wl1=Wl1, bl1=bl1, Wl=Wl, bl=bl)
    in_maps = _make_in_maps(prep, inp)
    res = bass_utils.run_bass_kernel_spmd(
        nc, in_maps, core_ids=list(range(N_CORES)))
    out = res.results[0]["out_head"]
    return np.ascontiguousarray(out.T.astype(np.float32))


# revision 23
# speedup vs baseline: 1.2114x; 1.0185x over previous
"""GCN (4x GCNConv + global mean pool + MLP head) on 8 Trainium2 NeuronCores.

Key algebraic restructuring (vs. naive 4x gather-aggregate):
  - Layer 1 aggregation is linear in the static input:  agg0 = (A~ x) @ W1,
    so A~ x is precomputed on the host and layer 1 on device is one dense
    matmul + ReLU. No exchange, no gathers.
  - Layer 4 has no ReLU between aggregation and mean-pool, so the pool
    collapses through the aggregation: pooled[g] = sum_m C[m,g] * h4[m]
    with C[m,g] = sum_{e: src=m, graph(dst)=g} w_e + selfw_m [graph(m)=g],
    host-built. No exchange, no gathers, aggregation rides the pool matmul;
    cross-core reduction is the existing AllReduce.
  - Only layers 2 and 3 exchange features: h (fp8) is AllGathered per slab,
    edge rows are fetched with per-(window,subtile) indirect DMAs, and the
    aggregation runs as fp8 DoubleRow matmuls against host-built fp8
    selection/scale matrices S resident in SBUF (edge + self-loop diag).

Node sharding: 100k nodes -> 8 cores x 98 windows x 128 slots via
capacity-constrained bin packing (<=512 in-edges per window, so exactly
4 edge subtiles per window). All feature exchange in fp8 e4m3 with f32
accumulation; weights/activations in bf16.
"""

import heapq

import numpy as np
import ml_dtypes

# ---------------------------------------------------------------- constants
N_NODES = 100000
N_EDGES = 400000
N_GRAPHS = 64
DIMS = [(512, 512), (512, 256), (256, 128), (128, 64)]
N_CORES = 8
P = 128
W_WINDOWS = 98
SLOTS = W_WINDOWS * P            # 12544 per core
SLOTS_ALL = SLOTS * N_CORES      # 100352
BF16 = ml_dtypes.bfloat16
F8 = ml_dtypes.float8_e4m3fn
SUB = 4                          # edge subtiles per window
NSUB = W_WINDOWS * SUB           # 392 per core
N_SLAB = 4                       # AllGather slabs per exchanged layer


def _slabs(w_windows):
    base = w_windows // N_SLAB
    rem = w_windows % N_SLAB
    out = []
    w0 = 0
    for s in range(N_SLAB):
        nwin = base + (1 if s < rem else 0)
        if nwin > 0:
            out.append((w0, nwin))
        w0 += nwin
    return out


# ---------------------------------------------------------------- host prep
def _pack_nodes(cost, sub_real):
    nb = N_CORES * W_WINDOWS
    cap = sub_real * P
    order = np.argsort(-cost, kind="stable")
    bin_load = np.zeros(nb, dtype=np.int64)
    bin_cnt = np.zeros(nb, dtype=np.int64)
    node_bin = np.full(len(cost), -1, dtype=np.int64)
    heap = [(0, b) for b in range(nb)]
    heapq.heapify(heap)
    stash = []
    for n in order:
        c = cost[n]
        stash.clear()
        placed = False
        while heap:
            load, b = heapq.heappop(heap)
            if bin_load[b] + c <= cap and bin_cnt[b] < P:
                bin_load[b] += c
                bin_cnt[b] += 1
                node_bin[n] = b
                if bin_cnt[b] < P:
                    heapq.heappush(heap, (bin_load[b], b))
                placed = True
                break
            elif bin_cnt[b] < P:
                stash.append((load, b))
        for it in stash:
            heapq.heappush(heap, it)
        if not placed:
            return None, None
    return node_bin, bin_load


def _preprocess(x, edge_index, batch):
    src = np.asarray(edge_index[0], dtype=np.int64)
    dst = np.asarray(edge_index[1], dtype=np.int64)
    batch = np.asarray(batch, dtype=np.int64)
    n = x.shape[0]

    indeg = np.bincount(dst, minlength=n).astype(np.int64)
    deg = indeg.astype(np.float64) + 1.0
    dinv = 1.0 / np.sqrt(deg)
    enorm = (dinv[src] * dinv[dst]).astype(np.float32)
    selfw = (dinv * dinv).astype(np.float32)

    node_bin, bin_load = _pack_nodes(indeg, SUB)
    assert node_bin is not None, "window packing failed at cap 512"

    nb = N_CORES * W_WINDOWS
    order = np.argsort(-bin_load, kind="stable")
    bin_core = np.empty(nb, dtype=np.int64)
    bin_win = np.empty(nb, dtype=np.int64)
    for i, b in enumerate(order):
        rnd, k = divmod(i, N_CORES)
        c = k if rnd % 2 == 0 else N_CORES - 1 - k
        bin_core[b] = c
        bin_win[b] = rnd

    node_core = bin_core[node_bin]
    node_win = bin_win[node_bin]
    gkey = node_core * W_WINDOWS + node_win
    sort_idx = np.argsort(gkey, kind="stable")
    gsorted = gkey[sort_idx]
    grp_start = np.searchsorted(gsorted, np.arange(nb))
    slot_in_win = np.empty(n, dtype=np.int64)
    slot_in_win[sort_idx] = np.arange(n) - grp_start[gsorted]
    assert slot_in_win.max() < P
    node_slot = node_core * SLOTS + node_win * P + slot_in_win

    # agout global row (slab-major layout so AG slabs are contiguous)
    slabs = _slabs(W_WINDOWS)
    win_slab = np.zeros(W_WINDOWS, dtype=np.int64)
    win_off = np.zeros(W_WINDOWS, dtype=np.int64)
    slab_off = np.zeros(N_SLAB, dtype=np.int64)
    slab_rows = np.zeros(N_SLAB, dtype=np.int64)
    off = 0
    for s, (w0, nwin) in enumerate(slabs):
        win_slab[w0:w0 + nwin] = s
        win_off[w0:w0 + nwin] = np.arange(nwin)
        slab_off[s] = off
        slab_rows[s] = nwin * P
        off += N_CORES * nwin * P
    node_grow = (slab_off[win_slab[node_win]]
                 + node_core * slab_rows[win_slab[node_win]]
                 + win_off[node_win] * P + slot_in_win)

    # ---- edge slot layout: per (core, window) 4 subtiles of 128
    e_core = node_core[dst]
    e_win = node_win[dst]
    e_pdst = slot_in_win[dst]

    ekey = e_core * W_WINDOWS + e_win
    es = np.argsort(ekey, kind="stable")
    eks = ekey[es]
    egrp_start = np.searchsorted(eks, np.arange(nb))
    e_rank = np.empty(len(ekey), dtype=np.int64)
    e_rank[es] = np.arange(len(ekey)) - egrp_start[eks]
    assert e_rank.max() < SUB * P, "window overflow"

    e_j = e_rank // P
    e_p = e_rank % P

    gidx = np.zeros((N_CORES, P, NSUB), dtype=np.int32)
    gidx[e_core, e_p, e_win * SUB + e_j] = node_grow[src].astype(np.int32)

    # host-built fp8 S tiles: edge subtiles + self-loop diagonals
    S8 = np.zeros((N_CORES, P, NSUB, P), dtype=np.float32)
    S8[e_core, e_p, e_win * SUB + e_j, e_pdst] = enorm
    S8 = S8.astype(F8).reshape(N_CORES, P, NSUB * P)
    selfS8 = np.zeros((N_CORES, P, W_WINDOWS, P), dtype=np.float32)
    selfS8[node_core, slot_in_win, node_win, slot_in_win] = selfw
    selfS8 = selfS8.astype(F8).reshape(N_CORES, P, W_WINDOWS * P)

    # ---- layer-1 host aggregation: X~ = A~ x  (then device does X~ @ W1)
    xt = x.astype(np.float32)
    ax = selfw[:, None] * xt
    np.add.at(ax, dst, enorm[:, None] * xt[src])
    x0 = np.zeros((N_CORES, SLOTS, xt.shape[1]), dtype=BF16)
    x0.reshape(SLOTS_ALL, -1)[node_slot] = ax.astype(BF16)
    x0 = np.ascontiguousarray(x0.transpose(0, 2, 1))

    # ---- layer-4 aggregation folded into the pool matrix:
    #   C[m, g] = sum_{e: src=m, graph(dst)=g} w_e + selfw_m [graph(m)=g]
    Cm = np.zeros((n, N_GRAPHS), dtype=np.float32)
    np.add.at(Cm, (src, batch[dst]), enorm)
    Cm[np.arange(n), batch] += selfw
    poolC = np.zeros((N_CORES, P, W_WINDOWS * N_GRAPHS), dtype=BF16)
    pc = (node_win * N_GRAPHS)[:, None] + np.arange(N_GRAPHS)[None, :]
    poolC[node_core[:, None], slot_in_win[:, None], pc] = Cm.astype(BF16)

    cnts = np.bincount(batch, minlength=N_GRAPHS).astype(np.float32)
    inv_cnt = (1.0 / np.maximum(cnts, 1.0)).reshape(N_GRAPHS, 1)

    return dict(gidx=gidx, S8=S8, selfS8=selfS8, x0=x0,
                poolC=poolC, inv_cnt=inv_cnt)


# ---------------------------------------------------------------- device IR
def build_program(has_bias, n_cores=N_CORES, w_windows=W_WINDOWS,
                  dims=DIMS, n_graphs=N_GRAPHS):
    from contextlib import ExitStack

    import concourse.bass as bass
    import concourse.tile as tile
    from concourse import bacc, mybir
    from concourse.masks import make_identity

    dt = mybir.dt
    f32, bf16, i32, f8 = dt.float32, dt.bfloat16, dt.int32, dt.float8e4
    AF = mybir.ActivationFunctionType
    ALU = mybir.AluOpType
    PM = mybir.MatmulPerfMode
    W = w_windows
    slots = W * P
    slots_all = slots * n_cores
    G = n_graphs
    d_last = dims[-1][1]
    nlay = len(dims)
    rg = [list(range(n_cores))]
    XL = (1, 2)  # layers whose aggregation needs exchange + gathers
    assert not has_bias[3], "C-pool fold requires b4 == 0"

    nc = bacc.Bacc("TRN2", target_bir_lowering=False, debug=False,
                   num_devices=n_cores)

    # ---- I/O
    x0 = nc.dram_tensor("x0", [dims[0][0], slots], bf16, kind="ExternalInput")
    gidx_d = nc.dram_tensor("gidx", [P, NSUB], i32, kind="ExternalInput")
    S_d = nc.dram_tensor("S8", [P, NSUB * P], f8, kind="ExternalInput")
    selfS_d = nc.dram_tensor("selfS8", [P, W * P], f8, kind="ExternalInput")
    W_d = [nc.dram_tensor(f"W{i+1}", [di, do], bf16, kind="ExternalInput")
           for i, (di, do) in enumerate(dims)]
    B_d = [nc.dram_tensor(f"B{i+1}", [P, do], f32, kind="ExternalInput")
           if has_bias[i] else None for i, (_, do) in enumerate(dims)]
    poolC_d = nc.dram_tensor("poolC", [P, W * G], bf16, kind="ExternalInput")
    Wl1_d = nc.dram_tensor("Wl1", [d_last, 32], f32, kind="ExternalInput")
    bl1_d = nc.dram_tensor("bl1", [32, 1], f32, kind="ExternalInput")
    Wl_d = nc.dram_tensor("Wl", [32, 2], f32, kind="ExternalInput")
    bl_d = nc.dram_tensor("bl", [2, 1], f32, kind="ExternalInput")
    invc_d = nc.dram_tensor("invc", [G, 1], f32, kind="ExternalInput")
    out_head = nc.dram_tensor("out_head", [2, G], f32, kind="ExternalOutput")

    # ---- internal DRAM (exchange only for layers in XL)
    agin = {l: nc.dram_tensor(f"agin{l}", [slots, dims[l][1]], f8)
            for l in XL}
    agout = {l: nc.dram_tensor(f"agout{l}", [slots_all, dims[l][1]], f8,
                               addr_space="Shared") for l in XL}
    feat = [nc.dram_tensor(f"feat{l}", [slots, do], bf16)
            for l, (_, do) in enumerate(dims[:-1])]
    pool_in = nc.dram_tensor("pool_in", [G, d_last], f32)
    pool_out = nc.dram_tensor("pool_out", [G, d_last], f32,
                              addr_space="Shared")

    with tile.TileContext(nc) as tc, ExitStack() as ctx:
        const = ctx.enter_context(tc.tile_pool(name="const", bufs=1))
        xt_pool = ctx.enter_context(tc.tile_pool(name="xt", bufs=8))
        g_pool = ctx.enter_context(tc.tile_pool(name="g", bufs=24))
        sg_pool = ctx.enter_context(tc.tile_pool(name="sg", bufs=6))
        pp_pool = ctx.enter_context(tc.tile_pool(name="pp", bufs=4))
        h_pool = ctx.enter_context(tc.tile_pool(name="h", bufs=8))
        psum_m = ctx.enter_context(tc.tile_pool(name="pm", bufs=3, space="PSUM"))
        psum_a = ctx.enter_context(tc.tile_pool(name="pa", bufs=4, space="PSUM"))
        psum_s = ctx.enter_context(tc.tile_pool(name="ps", bufs=1, space="PSUM"))

        # resident constants
        gidx_sb = const.tile([P, NSUB], i32, name="gidx_sb")
        nc.sync.dma_start(gidx_sb[:], gidx_d.ap())
        S_sb = const.tile([P, NSUB, P], f8, name="S_sb")
        nc.sync.dma_start(S_sb[:], S_d.ap().rearrange("p (q m) -> p q m", m=P))
        selfS_sb = const.tile([P, W, P], f8, name="selfS_sb")
        nc.sync.dma_start(selfS_sb[:],
                          selfS_d.ap().rearrange("p (w m) -> p w m", m=P))

        W_sb = []
        for l, (di, do) in enumerate(dims):
            ks = di // P
            t = const.tile([P, ks, do], bf16, name=f"W{l}_sb")
            nc.sync.dma_start(t[:], W_d[l].ap().rearrange(
                "(kt p) do -> p kt do", p=P))
            W_sb.append(t)
        B_sb = []
        for l, (_, do) in enumerate(dims):
            if has_bias[l]:
                t = const.tile([P, do], f32, name=f"B{l}_sb")
                nc.sync.dma_start(t[:], B_d[l].ap())
                B_sb.append(t)
            else:
                B_sb.append(None)

        Wl1_sb = const.tile([d_last, 32], f32, name="Wl1_sb")
        nc.sync.dma_start(Wl1_sb[:], Wl1_d.ap())
        bl1_sb = const.tile([32, 1], f32, name="bl1_sb")
        nc.sync.dma_start(bl1_sb[:], bl1_d.ap())
        Wl_sb = const.tile([32, 2], f32, name="Wl_sb")
        nc.sync.dma_start(Wl_sb[:], Wl_d.ap())
        bl_sb = const.tile([2, 1], f32, name="bl_sb")
        nc.sync.dma_start(bl_sb[:], bl_d.ap())
        invc_sb = const.tile([G, 1], f32, name="invc_sb")
        nc.sync.dma_start(invc_sb[:], invc_d.ap())
        iden = const.tile([G, G], f32, name="iden")
        make_identity(nc, iden[:])

        x0_v = x0.ap().rearrange("(kt p) s -> p kt s", p=P)
        slabs = _slabs(W)
        pool_ps = psum_s.tile([G, d_last], f32, name="pool_ps",
                              tag="ps_small")

        # ---- M: compute h_l[w] = feat[l-1][w] @ W_l
        # l == 0: X~ @ W1, straight to feat[0] (aggregation precomputed)
        # l in XL: cast to fp8, store to agin[l] for the exchange
        # l == 3: matmul C-pool immediately (aggregation folded into pool)
        def emit_m_window(l, w):
            di, do = dims[l]
            ks = di // P
            xt = xt_pool.tile([P, ks, P], bf16, tag="xt")
            if l == 0:
                nc.sync.dma_start(xt[:], x0_v[:, :, w * P:(w + 1) * P])
            else:
                nc.sync.dma_start_transpose(
                    xt[:], feat[l - 1].ap()[w * P:(w + 1) * P, :])
            ps = psum_m.tile([P, do], f32, tag="pm")
            for kt in range(ks):
                nc.tensor.matmul(ps[:], lhsT=xt[:, kt, :],
                                 rhs=W_sb[l][:, kt, :],
                                 start=(kt == 0), stop=(kt == ks - 1))
            if l == 0:
                if has_bias[0]:
                    nc.vector.tensor_tensor(out=ps[:], in0=ps[:],
                                            in1=B_sb[0][:], op=ALU.add)
                ft = h_pool.tile([P, do], bf16, tag="ft0")
                nc.scalar.activation(ft[:], ps[:], AF.Relu)
                nc.scalar.dma_start(feat[0].ap()[w * P:(w + 1) * P, :], ft[:])
            elif l in XL:
                hm = h_pool.tile([P, do], f8, tag="hm")
                nc.vector.tensor_copy(hm[:], ps[:])
                nc.scalar.dma_start(agin[l].ap()[w * P:(w + 1) * P, :], hm[:])
            else:  # l == 3: h4 window -> C-pool matmul (requires b4 == 0)
                h4 = h_pool.tile([P, do], bf16, tag="h4")
                nc.vector.tensor_copy(h4[:], ps[:])
                pt = pp_pool.tile([P, G], bf16, tag="pp")
                nc.scalar.dma_start(
                    pt[:], poolC_d.ap()[:, w * G:(w + 1) * G])
                nc.tensor.matmul(pool_ps[:], lhsT=pt[:], rhs=h4[:],
                                 start=(w == 0), stop=(w == W - 1))

        def emit_ag_slab(l, w0s, nwin, goff):
            rows = nwin * P
            nc.gpsimd.collective_compute(
                "AllGather", mybir.AluOpType.bypass, replica_groups=rg,
                ins=[agin[l].ap()[w0s * P:w0s * P + rows, :]],
                outs=[agout[l].ap()[goff:goff + n_cores * rows, :]])

        goffs = []
        goff = 0
        for (w0s, nwin) in slabs:
            goffs.append(goff)
            goff += n_cores * nwin * P
        slab_last = {w0s + nwin - 1: (si, w0s, nwin)
                     for si, (w0s, nwin) in enumerate(slabs)}

        def emit_m_and_ag(l1, wm):
            emit_m_window(l1, wm)
            if l1 in XL and wm in slab_last:
                si, w0s, nwin = slab_last[wm]
                emit_ag_slab(l1, w0s, nwin, goffs[si])

        # ---- A: aggregate layer l in XL (gather + fp8 DoubleRow matmuls)
        def emit_a_window(l, w):
            do = dims[l][1]
            g = g_pool.tile([P, SUB, do], f8, tag="g")
            for j in range(SUB):
                nc.gpsimd.indirect_dma_start(
                    out=g[:, j, :], out_offset=None, in_=agout[l].ap(),
                    in_offset=bass.IndirectOffsetOnAxis(
                        ap=gidx_sb[:, w * SUB + j:w * SUB + j + 1],
                        axis=0))
            gs = sg_pool.tile([P, do], f8, tag="gs")
            nc.sync.dma_start(gs[:], agin[l].ap()[w * P:(w + 1) * P, :])
            ps = psum_a.tile([P, do], f32, tag="pa")
            for k in range(2):
                q = w * SUB + 2 * k
                nc.tensor.matmul(ps[:], lhsT=S_sb[:, q:q + 2, :],
                                 rhs=g[:, 2 * k:2 * k + 2, :],
                                 start=(k == 0), stop=False,
                                 perf_mode=PM.DoubleRow)
            nc.tensor.matmul(ps[:], lhsT=selfS_sb[:, w, :], rhs=gs[:],
                             start=False, stop=True)
            if has_bias[l]:
                nc.vector.tensor_tensor(out=ps[:], in0=ps[:],
                                        in1=B_sb[l][:], op=ALU.add)
            ft = h_pool.tile([P, do], bf16, tag=f"ft{do}")
            nc.scalar.activation(ft[:], ps[:], AF.Relu)
            nc.scalar.dma_start(feat[l].ap()[w * P:(w + 1) * P, :], ft[:])

        # ---------------- schedule
        # L0 (dense) with M(1) + AG(1) trailing right behind it
        LAG = 2
        for w in range(W):
            emit_m_window(0, w)
            if w >= LAG:
                emit_m_and_ag(1, w - LAG)
        for wm in range(W - LAG, W):
            emit_m_and_ag(1, wm)
        # A(1) with M(2)/AG(2) trailing; A(2) with M(3)+pool trailing
        for l in XL:
            for w in range(W):
                emit_a_window(l, w)
                if w >= LAG:
                    emit_m_and_ag(l + 1, w - LAG)
            for wm in range(W - LAG, W):
                emit_m_and_ag(l + 1, wm)

        # ---------------- mean pool tail + head
        pool_sb = const.tile([G, d_last], f32, name="pool_sb")
        nc.vector.tensor_copy(pool_sb[:], pool_ps[:])
        nc.sync.dma_start(pool_in.ap(), pool_sb[:])
        nc.gpsimd.collective_compute(
            "AllReduce", mybir.AluOpType.add, replica_groups=rg,
            ins=[pool_in.ap()], outs=[pool_out.ap()])
        psum_sb = const.tile([G, d_last], f32, name="psum_sb")
        nc.sync.dma_start(psum_sb[:], pool_out.ap())
        pooled = const.tile([G, d_last], f32, name="pooled")
        nc.vector.tensor_scalar_mul(pooled[:], psum_sb[:], invc_sb[:, :1])

        pt_ps = psum_s.tile([d_last, G], f32, name="pt_ps", tag="ps_small")
        nc.tensor.transpose(pt_ps[:], pooled[:], iden[:])
        pt = const.tile([d_last, G], f32, name="pt")
        nc.vector.tensor_copy(pt[:], pt_ps[:])
        ps1 = psum_s.tile([32, G], f32, name="ps1", tag="ps_small")
        nc.tensor.matmul(ps1[:], lhsT=Wl1_sb[:], rhs=pt[:])
        h1 = const.tile([32, G], f32, name="h1")
        nc.scalar.activation(h1[:], ps1[:], AF.Relu, bias=bl1_sb[:, :1])
        ps2 = psum_s.tile([2, G], f32, name="ps2", tag="ps_small")
        nc.tensor.matmul(ps2[:], lhsT=Wl_sb[:], rhs=h1[:])
        oh = const.tile([2, G], f32, name="oh")
        nc.vector.tensor_scalar_add(oh[:], ps2[:], bl_sb[:, :1])
        nc.sync.dma_start(out_head.ap(), oh[:])

    nc.compile()
    return nc


# ---------------------------------------------------------------- entry
_CACHE = {}


def _make_in_maps(prep, inp):
    Ws = [np.asarray(inp[f"W{i+1}"]) for i in range(4)]
    bs = [np.asarray(inp[f"b{i+1}"]) for i in range(4)]
    has_bias = tuple(bool(np.any(b != 0)) for b in bs)
    in_maps = []
    for c in range(N_CORES):
        m = dict(
            x0=prep["x0"][c],
            gidx=prep["gidx"][c], S8=prep["S8"][c], selfS8=prep["selfS8"][c],
            poolC=prep["poolC"][c], invc=prep["inv_cnt"],
            Wl1=np.asarray(inp["Wl1"], np.float32),
            bl1=np.asarray(inp["bl1"], np.float32).reshape(-1, 1),
            Wl=np.asarray(inp["Wl"], np.float32),
            bl=np.asarray(inp["bl"], np.float32).reshape(-1, 1),
        )
        for i, (wm, bv) in enumerate(zip(Ws, bs)):
            m[f"W{i+1}"] = wm.astype(BF16)
            if has_bias[i]:
                m[f"B{i+1}"] = np.broadcast_to(
                    np.asarray(bv, np.float32), (P, len(bv))).copy()
        in_maps.append(m)
    return in_maps


def kernel(x, edge_index, batch, W1, b1, W2, b2, W3, b3, W4, b4,
           Wl1, bl1, Wl, bl):
    from concourse import bass_utils

    x = np.asarray(x)
    prep = _preprocess(x, np.asarray(edge_index), np.asarray(batch))
    bs = [np.asarray(b) for b in (b1, b2, b3, b4)]
    has_bias = tuple(bool(np.any(b != 0)) for b in bs)

    if has_bias not in _CACHE:
        _CACHE[has_bias] = build_program(has_bias)
    nc = _CACHE[has_bias]

    inp = dict(W1=W1, b1=b1, W2=W2, b2=b2, W3=W3, b3=b3, W4=W4, b4=b4,
               Wl1=Wl1, bl1=bl1, Wl=Wl, bl=bl)
    in_maps = _make_in_maps(prep, inp)
    res = bass_utils.run_bass_kernel_spmd(
        nc, in_maps, core_ids=list(range(N_CORES)))
    out = res.results[0]["out_head"]
    return np.ascontiguousarray(out.T.astype(np.float32))


# revision 24
# speedup vs baseline: 1.2168x; 1.0044x over previous
"""GCN (4x GCNConv + global mean pool + MLP head) on 8 Trainium2 NeuronCores.

Key algebraic restructuring (vs. naive 4x gather-aggregate):
  - Layer 1 aggregation is linear in the static input:  agg0 = (A~ x) @ W1,
    so A~ x is precomputed on the host and layer 1 on device is one dense
    matmul + ReLU. No exchange, no gathers.
  - Layer 4 has no ReLU between aggregation and mean-pool, so the pool
    collapses through the aggregation: pooled[g] = sum_m C[m,g] * h4[m]
    with C[m,g] = sum_{e: src=m, graph(dst)=g} w_e + selfw_m [graph(m)=g],
    host-built. No exchange, no gathers, aggregation rides the pool matmul;
    cross-core reduction is the existing AllReduce.
  - Only layers 2 and 3 exchange features: h (fp8) is AllGathered per slab,
    edge rows are fetched with per-(window,subtile) indirect DMAs, and the
    aggregation runs as fp8 DoubleRow matmuls against host-built fp8
    selection/scale matrices S resident in SBUF (edge + self-loop diag).

Node sharding: 100k nodes -> 8 cores x 98 windows x 128 slots via
capacity-constrained bin packing (<=512 in-edges per window, so exactly
4 edge subtiles per window). All feature exchange in fp8 e4m3 with f32
accumulation; weights/activations in bf16.
"""

import heapq

import numpy as np
import ml_dtypes

# ---------------------------------------------------------------- constants
N_NODES = 100000
N_EDGES = 400000
N_GRAPHS = 64
DIMS = [(512, 512), (512, 256), (256, 128), (128, 64)]
N_CORES = 8
P = 128
W_WINDOWS = 98
SLOTS = W_WINDOWS * P            # 12544 per core
SLOTS_ALL = SLOTS * N_CORES      # 100352
BF16 = ml_dtypes.bfloat16
F8 = ml_dtypes.float8_e4m3fn
SUB = 4                          # edge subtiles per window
NSUB = W_WINDOWS * SUB           # 392 per core
N_SLAB = 4                       # AllGather slabs per exchanged layer


def _slabs(w_windows):
    base = w_windows // N_SLAB
    rem = w_windows % N_SLAB
    out = []
    w0 = 0
    for s in range(N_SLAB):
        nwin = base + (1 if s < rem else 0)
        if nwin > 0:
            out.append((w0, nwin))
        w0 += nwin
    return out


# ---------------------------------------------------------------- host prep
def _pack_nodes(cost, sub_real):
    nb = N_CORES * W_WINDOWS
    cap = sub_real * P
    order = np.argsort(-cost, kind="stable")
    bin_load = np.zeros(nb, dtype=np.int64)
    bin_cnt = np.zeros(nb, dtype=np.int64)
    node_bin = np.full(len(cost), -1, dtype=np.int64)
    heap = [(0, b) for b in range(nb)]
    heapq.heapify(heap)
    stash = []
    for n in order:
        c = cost[n]
        stash.clear()
        placed = False
        while heap:
            load, b = heapq.heappop(heap)
            if bin_load[b] + c <= cap and bin_cnt[b] < P:
                bin_load[b] += c
                bin_cnt[b] += 1
                node_bin[n] = b
                if bin_cnt[b] < P:
                    heapq.heappush(heap, (bin_load[b], b))
                placed = True
                break
            elif bin_cnt[b] < P:
                stash.append((load, b))
        for it in stash:
            heapq.heappush(heap, it)
        if not placed:
            return None, None
    return node_bin, bin_load


def _preprocess(x, edge_index, batch):
    src = np.asarray(edge_index[0], dtype=np.int64)
    dst = np.asarray(edge_index[1], dtype=np.int64)
    batch = np.asarray(batch, dtype=np.int64)
    n = x.shape[0]

    indeg = np.bincount(dst, minlength=n).astype(np.int64)
    deg = indeg.astype(np.float64) + 1.0
    dinv = 1.0 / np.sqrt(deg)
    enorm = (dinv[src] * dinv[dst]).astype(np.float32)
    selfw = (dinv * dinv).astype(np.float32)

    node_bin, bin_load = _pack_nodes(indeg, SUB)
    assert node_bin is not None, "window packing failed at cap 512"

    nb = N_CORES * W_WINDOWS
    order = np.argsort(-bin_load, kind="stable")
    bin_core = np.empty(nb, dtype=np.int64)
    bin_win = np.empty(nb, dtype=np.int64)
    for i, b in enumerate(order):
        rnd, k = divmod(i, N_CORES)
        c = k if rnd % 2 == 0 else N_CORES - 1 - k
        bin_core[b] = c
        bin_win[b] = rnd

    node_core = bin_core[node_bin]
    node_win = bin_win[node_bin]
    gkey = node_core * W_WINDOWS + node_win
    sort_idx = np.argsort(gkey, kind="stable")
    gsorted = gkey[sort_idx]
    grp_start = np.searchsorted(gsorted, np.arange(nb))
    slot_in_win = np.empty(n, dtype=np.int64)
    slot_in_win[sort_idx] = np.arange(n) - grp_start[gsorted]
    assert slot_in_win.max() < P
    node_slot = node_core * SLOTS + node_win * P + slot_in_win

    # agout global row (slab-major layout so AG slabs are contiguous)
    slabs = _slabs(W_WINDOWS)
    win_slab = np.zeros(W_WINDOWS, dtype=np.int64)
    win_off = np.zeros(W_WINDOWS, dtype=np.int64)
    slab_off = np.zeros(N_SLAB, dtype=np.int64)
    slab_rows = np.zeros(N_SLAB, dtype=np.int64)
    off = 0
    for s, (w0, nwin) in enumerate(slabs):
        win_slab[w0:w0 + nwin] = s
        win_off[w0:w0 + nwin] = np.arange(nwin)
        slab_off[s] = off
        slab_rows[s] = nwin * P
        off += N_CORES * nwin * P
    node_grow = (slab_off[win_slab[node_win]]
                 + node_core * slab_rows[win_slab[node_win]]
                 + win_off[node_win] * P + slot_in_win)

    # ---- edge slot layout: per (core, window) 4 subtiles of 128
    e_core = node_core[dst]
    e_win = node_win[dst]
    e_pdst = slot_in_win[dst]

    ekey = e_core * W_WINDOWS + e_win
    es = np.argsort(ekey, kind="stable")
    eks = ekey[es]
    egrp_start = np.searchsorted(eks, np.arange(nb))
    e_rank = np.empty(len(ekey), dtype=np.int64)
    e_rank[es] = np.arange(len(ekey)) - egrp_start[eks]
    assert e_rank.max() < SUB * P, "window overflow"

    e_j = e_rank // P
    e_p = e_rank % P

    gidx = np.zeros((N_CORES, P, NSUB), dtype=np.int32)
    gidx[e_core, e_p, e_win * SUB + e_j] = node_grow[src].astype(np.int32)

    # host-built fp8 S tiles: edge subtiles + self-loop diagonals
    S8 = np.zeros((N_CORES, P, NSUB, P), dtype=np.float32)
    S8[e_core, e_p, e_win * SUB + e_j, e_pdst] = enorm
    S8 = S8.astype(F8).reshape(N_CORES, P, NSUB * P)
    selfS8 = np.zeros((N_CORES, P, W_WINDOWS, P), dtype=np.float32)
    selfS8[node_core, slot_in_win, node_win, slot_in_win] = selfw
    selfS8 = selfS8.astype(F8).reshape(N_CORES, P, W_WINDOWS * P)

    # ---- layer-1 host aggregation: X~ = A~ x  (then device does X~ @ W1)
    xt = x.astype(np.float32)
    ax = selfw[:, None] * xt
    np.add.at(ax, dst, enorm[:, None] * xt[src])
    x0 = np.zeros((N_CORES, SLOTS, xt.shape[1]), dtype=BF16)
    x0.reshape(SLOTS_ALL, -1)[node_slot] = ax.astype(BF16)
    x0 = np.ascontiguousarray(x0.transpose(0, 2, 1))

    # ---- layer-4 aggregation folded into the pool matrix:
    #   C[m, g] = sum_{e: src=m, graph(dst)=g} w_e + selfw_m [graph(m)=g]
    Cm = np.zeros((n, N_GRAPHS), dtype=np.float32)
    np.add.at(Cm, (src, batch[dst]), enorm)
    Cm[np.arange(n), batch] += selfw
    poolC = np.zeros((N_CORES, P, W_WINDOWS * N_GRAPHS), dtype=BF16)
    pc = (node_win * N_GRAPHS)[:, None] + np.arange(N_GRAPHS)[None, :]
    poolC[node_core[:, None], slot_in_win[:, None], pc] = Cm.astype(BF16)

    cnts = np.bincount(batch, minlength=N_GRAPHS).astype(np.float32)
    inv_cnt = (1.0 / np.maximum(cnts, 1.0)).reshape(N_GRAPHS, 1)

    return dict(gidx=gidx, S8=S8, selfS8=selfS8, x0=x0,
                poolC=poolC, inv_cnt=inv_cnt)


# ---------------------------------------------------------------- device IR
def build_program(has_bias, n_cores=N_CORES, w_windows=W_WINDOWS,
                  dims=DIMS, n_graphs=N_GRAPHS):
    from contextlib import ExitStack

    import concourse.bass as bass
    import concourse.tile as tile
    from concourse import bacc, mybir
    from concourse.masks import make_identity

    dt = mybir.dt
    f32, bf16, i32, f8 = dt.float32, dt.bfloat16, dt.int32, dt.float8e4
    AF = mybir.ActivationFunctionType
    ALU = mybir.AluOpType
    PM = mybir.MatmulPerfMode
    W = w_windows
    slots = W * P
    slots_all = slots * n_cores
    G = n_graphs
    d_last = dims[-1][1]
    nlay = len(dims)
    rg = [list(range(n_cores))]
    XL = (1, 2)  # layers whose aggregation needs exchange + gathers
    assert not has_bias[3], "C-pool fold requires b4 == 0"

    nc = bacc.Bacc("TRN2", target_bir_lowering=False, debug=False,
                   num_devices=n_cores)

    # ---- I/O
    x0 = nc.dram_tensor("x0", [dims[0][0], slots], bf16, kind="ExternalInput")
    gidx_d = nc.dram_tensor("gidx", [P, NSUB], i32, kind="ExternalInput")
    S_d = nc.dram_tensor("S8", [P, NSUB * P], f8, kind="ExternalInput")
    selfS_d = nc.dram_tensor("selfS8", [P, W * P], f8, kind="ExternalInput")
    W_d = [nc.dram_tensor(f"W{i+1}", [di, do], bf16, kind="ExternalInput")
           for i, (di, do) in enumerate(dims)]
    B_d = [nc.dram_tensor(f"B{i+1}", [P, do], f32, kind="ExternalInput")
           if has_bias[i] else None for i, (_, do) in enumerate(dims)]
    poolC_d = nc.dram_tensor("poolC", [P, W * G], bf16, kind="ExternalInput")
    Wl1_d = nc.dram_tensor("Wl1", [d_last, 32], f32, kind="ExternalInput")
    bl1_d = nc.dram_tensor("bl1", [32, 1], f32, kind="ExternalInput")
    Wl_d = nc.dram_tensor("Wl", [32, 2], f32, kind="ExternalInput")
    bl_d = nc.dram_tensor("bl", [2, 1], f32, kind="ExternalInput")
    invc_d = nc.dram_tensor("invc", [G, 1], f32, kind="ExternalInput")
    out_head = nc.dram_tensor("out_head", [2, G], f32, kind="ExternalOutput")

    # ---- internal DRAM (exchange only for layers in XL)
    agin = {l: nc.dram_tensor(f"agin{l}", [slots, dims[l][1]], f8)
            for l in XL}
    agout = {l: nc.dram_tensor(f"agout{l}", [slots_all, dims[l][1]], f8,
                               addr_space="Shared") for l in XL}
    feat = [nc.dram_tensor(f"feat{l}", [slots, do], bf16)
            for l, (_, do) in enumerate(dims[:-1])]
    pool_in = nc.dram_tensor("pool_in", [G, d_last], f32)
    pool_out = nc.dram_tensor("pool_out", [G, d_last], f32,
                              addr_space="Shared")

    with tile.TileContext(nc) as tc, ExitStack() as ctx:
        const = ctx.enter_context(tc.tile_pool(name="const", bufs=1))
        xt_pool = ctx.enter_context(tc.tile_pool(name="xt", bufs=8))
        g_pool = ctx.enter_context(tc.tile_pool(name="g", bufs=24))
        sg_pool = ctx.enter_context(tc.tile_pool(name="sg", bufs=6))
        pp_pool = ctx.enter_context(tc.tile_pool(name="pp", bufs=4))
        h_pool = ctx.enter_context(tc.tile_pool(name="h", bufs=8))
        psum_m = ctx.enter_context(tc.tile_pool(name="pm", bufs=3, space="PSUM"))
        psum_a = ctx.enter_context(tc.tile_pool(name="pa", bufs=4, space="PSUM"))
        psum_s = ctx.enter_context(tc.tile_pool(name="ps", bufs=1, space="PSUM"))

        # resident constants
        gidx_sb = const.tile([P, NSUB], i32, name="gidx_sb")
        nc.sync.dma_start(gidx_sb[:], gidx_d.ap())
        S_sb = const.tile([P, NSUB, P], f8, name="S_sb")
        nc.sync.dma_start(S_sb[:], S_d.ap().rearrange("p (q m) -> p q m", m=P))
        selfS_sb = const.tile([P, W, P], f8, name="selfS_sb")
        nc.sync.dma_start(selfS_sb[:],
                          selfS_d.ap().rearrange("p (w m) -> p w m", m=P))

        W_sb = []
        for l, (di, do) in enumerate(dims):
            ks = di // P
            t = const.tile([P, ks, do], bf16, name=f"W{l}_sb")
            nc.sync.dma_start(t[:], W_d[l].ap().rearrange(
                "(kt p) do -> p kt do", p=P))
            W_sb.append(t)
        B_sb = []
        for l, (_, do) in enumerate(dims):
            if has_bias[l]:
                t = const.tile([P, do], f32, name=f"B{l}_sb")
                nc.sync.dma_start(t[:], B_d[l].ap())
                B_sb.append(t)
            else:
                B_sb.append(None)

        Wl1_sb = const.tile([d_last, 32], f32, name="Wl1_sb")
        nc.sync.dma_start(Wl1_sb[:], Wl1_d.ap())
        bl1_sb = const.tile([32, 1], f32, name="bl1_sb")
        nc.sync.dma_start(bl1_sb[:], bl1_d.ap())
        Wl_sb = const.tile([32, 2], f32, name="Wl_sb")
        nc.sync.dma_start(Wl_sb[:], Wl_d.ap())
        bl_sb = const.tile([2, 1], f32, name="bl_sb")
        nc.sync.dma_start(bl_sb[:], bl_d.ap())
        invc_sb = const.tile([G, 1], f32, name="invc_sb")
        nc.sync.dma_start(invc_sb[:], invc_d.ap())
        poolC_sb = const.tile([P, W * G], bf16, name="poolC_sb")
        nc.sync.dma_start(poolC_sb[:], poolC_d.ap())
        iden = const.tile([G, G], f32, name="iden")
        make_identity(nc, iden[:])

        x0_v = x0.ap().rearrange("(kt p) s -> p kt s", p=P)
        slabs = _slabs(W)
        pool_ps = psum_s.tile([G, d_last], f32, name="pool_ps",
                              tag="ps_small")

        # ---- M: compute h_l[w] = feat[l-1][w] @ W_l
        # l == 0: X~ @ W1, straight to feat[0] (aggregation precomputed)
        # l in XL: cast to fp8, store to agin[l] for the exchange
        # l == 3: matmul C-pool immediately (aggregation folded into pool)
        def emit_m_window(l, w):
            di, do = dims[l]
            ks = di // P
            xt = xt_pool.tile([P, ks, P], bf16, tag="xt")
            if l == 0:
                nc.sync.dma_start(xt[:], x0_v[:, :, w * P:(w + 1) * P])
            else:
                nc.sync.dma_start_transpose(
                    xt[:], feat[l - 1].ap()[w * P:(w + 1) * P, :])
            ps = psum_m.tile([P, do], f32, tag="pm")
            for kt in range(ks):
                nc.tensor.matmul(ps[:], lhsT=xt[:, kt, :],
                                 rhs=W_sb[l][:, kt, :],
                                 start=(kt == 0), stop=(kt == ks - 1))
            if l == 0:
                if has_bias[0]:
                    nc.vector.tensor_tensor(out=ps[:], in0=ps[:],
                                            in1=B_sb[0][:], op=ALU.add)
                ft = h_pool.tile([P, do], bf16, tag="ft0")
                nc.scalar.activation(ft[:], ps[:], AF.Relu)
                nc.scalar.dma_start(feat[0].ap()[w * P:(w + 1) * P, :], ft[:])
            elif l in XL:
                hm = h_pool.tile([P, do], f8, tag="hm")
                nc.vector.tensor_copy(hm[:], ps[:])
                nc.scalar.dma_start(agin[l].ap()[w * P:(w + 1) * P, :], hm[:])
            else:  # l == 3: h4 window -> C-pool matmul (requires b4 == 0)
                h4 = h_pool.tile([P, do], bf16, tag="h4")
                nc.vector.tensor_copy(h4[:], ps[:])
                nc.tensor.matmul(pool_ps[:],
                                 lhsT=poolC_sb[:, w * G:(w + 1) * G],
                                 rhs=h4[:],
                                 start=(w == 0), stop=(w == W - 1))

        def emit_ag_slab(l, w0s, nwin, goff):
            rows = nwin * P
            nc.gpsimd.collective_compute(
                "AllGather", mybir.AluOpType.bypass, replica_groups=rg,
                ins=[agin[l].ap()[w0s * P:w0s * P + rows, :]],
                outs=[agout[l].ap()[goff:goff + n_cores * rows, :]])

        goffs = []
        goff = 0
        for (w0s, nwin) in slabs:
            goffs.append(goff)
            goff += n_cores * nwin * P
        slab_last = {w0s + nwin - 1: (si, w0s, nwin)
                     for si, (w0s, nwin) in enumerate(slabs)}

        def emit_m_and_ag(l1, wm):
            emit_m_window(l1, wm)
            if l1 in XL and wm in slab_last:
                si, w0s, nwin = slab_last[wm]
                emit_ag_slab(l1, w0s, nwin, goffs[si])

        # ---- A: aggregate layer l in XL (gather + fp8 DoubleRow matmuls)
        def emit_a_window(l, w):
            do = dims[l][1]
            g = g_pool.tile([P, SUB, do], f8, tag="g")
            for j in range(SUB):
                nc.gpsimd.indirect_dma_start(
                    out=g[:, j, :], out_offset=None, in_=agout[l].ap(),
                    in_offset=bass.IndirectOffsetOnAxis(
                        ap=gidx_sb[:, w * SUB + j:w * SUB + j + 1],
                        axis=0))
            gs = sg_pool.tile([P, do], f8, tag="gs")
            nc.sync.dma_start(gs[:], agin[l].ap()[w * P:(w + 1) * P, :])
            ps = psum_a.tile([P, do], f32, tag="pa")
            for k in range(2):
                q = w * SUB + 2 * k
                nc.tensor.matmul(ps[:], lhsT=S_sb[:, q:q + 2, :],
                                 rhs=g[:, 2 * k:2 * k + 2, :],
                                 start=(k == 0), stop=False,
                                 perf_mode=PM.DoubleRow)
            nc.tensor.matmul(ps[:], lhsT=selfS_sb[:, w, :], rhs=gs[:],
                             start=False, stop=True)
            if has_bias[l]:
                nc.vector.tensor_tensor(out=ps[:], in0=ps[:],
                                        in1=B_sb[l][:], op=ALU.add)
            ft = h_pool.tile([P, do], bf16, tag=f"ft{do}")
            nc.scalar.activation(ft[:], ps[:], AF.Relu)
            nc.scalar.dma_start(feat[l].ap()[w * P:(w + 1) * P, :], ft[:])

        # ---------------- schedule
        # L0 (dense) with M(1) + AG(1) trailing right behind it
        LAG = 2
        for w in range(W):
            emit_m_window(0, w)
            if w >= LAG:
                emit_m_and_ag(1, w - LAG)
        for wm in range(W - LAG, W):
            emit_m_and_ag(1, wm)
        # A(1) with M(2)/AG(2) trailing; A(2) with M(3)+pool trailing
        for l in XL:
            for w in range(W):
                emit_a_window(l, w)
                if w >= LAG:
                    emit_m_and_ag(l + 1, w - LAG)
            for wm in range(W - LAG, W):
                emit_m_and_ag(l + 1, wm)

        # ---------------- mean pool tail + head
        pool_sb = const.tile([G, d_last], f32, name="pool_sb")
        nc.vector.tensor_copy(pool_sb[:], pool_ps[:])
        nc.sync.dma_start(pool_in.ap(), pool_sb[:])
        nc.gpsimd.collective_compute(
            "AllReduce", mybir.AluOpType.add, replica_groups=rg,
            ins=[pool_in.ap()], outs=[pool_out.ap()])
        psum_sb = const.tile([G, d_last], f32, name="psum_sb")
        nc.sync.dma_start(psum_sb[:], pool_out.ap())
        pooled = const.tile([G, d_last], f32, name="pooled")
        nc.vector.tensor_scalar_mul(pooled[:], psum_sb[:], invc_sb[:, :1])

        pt_ps = psum_s.tile([d_last, G], f32, name="pt_ps", tag="ps_small")
        nc.tensor.transpose(pt_ps[:], pooled[:], iden[:])
        pt = const.tile([d_last, G], f32, name="pt")
        nc.vector.tensor_copy(pt[:], pt_ps[:])
        ps1 = psum_s.tile([32, G], f32, name="ps1", tag="ps_small")
        nc.tensor.matmul(ps1[:], lhsT=Wl1_sb[:], rhs=pt[:])
        h1 = const.tile([32, G], f32, name="h1")
        nc.scalar.activation(h1[:], ps1[:], AF.Relu, bias=bl1_sb[:, :1])
        ps2 = psum_s.tile([2, G], f32, name="ps2", tag="ps_small")
        nc.tensor.matmul(ps2[:], lhsT=Wl_sb[:], rhs=h1[:])
        oh = const.tile([2, G], f32, name="oh")
        nc.vector.tensor_scalar_add(oh[:], ps2[:], bl_sb[:, :1])
        nc.sync.dma_start(out_head.ap(), oh[:])

    nc.compile()
    return nc


# ---------------------------------------------------------------- entry
_CACHE = {}


def _make_in_maps(prep, inp):
    Ws = [np.asarray(inp[f"W{i+1}"]) for i in range(4)]
    bs = [np.asarray(inp[f"b{i+1}"]) for i in range(4)]
    has_bias = tuple(bool(np.any(b != 0)) for b in bs)
    in_maps = []
    for c in range(N_CORES):
        m = dict(
            x0=prep["x0"][c],
            gidx=prep["gidx"][c], S8=prep["S8"][c], selfS8=prep["selfS8"][c],
            poolC=prep["poolC"][c], invc=prep["inv_cnt"],
            Wl1=np.asarray(inp["Wl1"], np.float32),
            bl1=np.asarray(inp["bl1"], np.float32).reshape(-1, 1),
            Wl=np.asarray(inp["Wl"], np.float32),
            bl=np.asarray(inp["bl"], np.float32).reshape(-1, 1),
        )
        for i, (wm, bv) in enumerate(zip(Ws, bs)):
            m[f"W{i+1}"] = wm.astype(BF16)
            if has_bias[i]:
                m[f"B{i+1}"] = np.broadcast_to(
                    np.asarray(bv, np.float32), (P, len(bv))).copy()
        in_maps.append(m)
    return in_maps


def kernel(x, edge_index, batch, W1, b1, W2, b2, W3, b3, W4, b4,
           Wl1, bl1, Wl, bl):
    from concourse import bass_utils

    x = np.asarray(x)
    prep = _preprocess(x, np.asarray(edge_index), np.asarray(batch))
    bs = [np.asarray(b) for b in (b1, b2, b3, b4)]
    has_bias = tuple(bool(np.any(b != 0)) for b in bs)

    if has_bias not in _CACHE:
        _CACHE[has_bias] = build_program(has_bias)
    nc = _CACHE[has_bias]

    inp = dict(W1=W1, b1=b1, W2=W2, b2=b2, W3=W3, b3=b3, W4=W4, b4=b4,
               Wl1=Wl1, bl1=bl1, Wl=Wl, bl=bl)
    in_maps = _make_in_maps(prep, inp)
    res = bass_utils.run_bass_kernel_spmd(
        nc, in_maps, core_ids=list(range(N_CORES)))
    out = res.results[0]["out_head"]
    return np.ascontiguousarray(out.T.astype(np.float32))
